# revision 2
# baseline (speedup 1.0000x reference)
"""Trainium2 Bass kernel for nn_BiLSTMModel (2-layer BiLSTM, B=1024 T=256 D=5 H=64).

V2: pure batch-DP over 8 cores (128 samples/core), restructured for engine
overlap:
 - two phase-shifted chains (batch halves of 64) per layer
 - comb_rev storage: col t = [h_f(t) ; h_b(T-1-t)] -> fwd+bwd recurrent matmul
   fuses into ONE block-diag matmul per gate, and the h-write is ONE DVE op
 - bias folded into the x-projection via a ones row (no bias matmul in l0);
   l1 bias via one small K=4 mask matmul (prefilled, off-chain)
 - psum bank per step [128, 512] shared by both chains: col = g*128 + c*64 + b
   prefill (x-proj / l1 in-proj) uses N=128 matmuls covering both chains
 - elementwise split: DVE does pt + s (STT), Pool (gpsimd) does r + h (plain TT;
   Pool has no scalar_tensor_tensor opcode)
 - transformed cell (s = 2c; g-gate rows of W/b scaled 2x so all four gate
   activations are one sigmoid; state nonlinearity via Tanh(0.5*s) so the
   h-write is a plain multiply):
     z  = W@x|h (+b)        -> psum f32
     u  = sigmoid(z)        -> gates ACT, per chain [128, 256] strided read
     pt = (u_g - 0.5)*u_i   -> DVE STT
     r  = u_f*s_old         -> pool TT
     s  = 4*pt + r          -> DVE STT
     th = tanh(0.5*s)       -> state ACT [128, 64] (= tanh(c))
     h  = th*u_o            -> pool TT (full-scale h; no 2x weight folds)
"""
import os
import numpy as np

import concourse.bacc as bacc
import concourse.bass as bass
import concourse.mybir as mybir
import concourse.tile as tile
from concourse.bass_utils import run_bass_kernel_spmd

H = 64
B = 128          # per-core batch
BC = 64          # per-chain batch
NCORES = 8
FULL_T = 256

F16 = mybir.dt.float16
F32 = mybir.dt.float32
AF = mybir.ActivationFunctionType
ALU = mybir.AluOpType


# ---------------------------------------------------------------- host packing

def _eff_dir(w_ih, w_hh, b_ih, b_hh):
    """Transformed-cell effective weights (float64 math). Gate order i,f,g,o.
    Only the g-gate rows are scaled 2x (tanh(g) = 2*sig(2g)-1); h is kept at
    full scale (state nonlinearity = tanh), so no other folds."""
    Wi = np.asarray(w_ih, np.float64).copy()
    Wh = np.asarray(w_hh, np.float64).copy()
    b = (np.asarray(b_ih, np.float64) + np.asarray(b_hh, np.float64)).copy()
    g = slice(2 * H, 3 * H)
    Wi = Wi.copy()
    Wi[g] *= 2.0
    Wh[g] *= 2.0
    b[g] *= 2.0
    return Wi, Wh, b


def make_core_inputs(inputs, T):
    Wif, Whf, bf = _eff_dir(inputs["w_ih_l0"], inputs["w_hh_l0"],
                            inputs["b_ih_l0"], inputs["b_hh_l0"])
    Wib, Whb, bb = _eff_dir(inputs["w_ih_l0r"], inputs["w_hh_l0r"],
                            inputs["b_ih_l0r"], inputs["b_hh_l0r"])
    Wif1, Whf1, bf1 = _eff_dir(inputs["w_ih_l1"], inputs["w_hh_l1"],
                               inputs["b_ih_l1"], inputs["b_hh_l1"])
    Wib1, Whb1, bb1 = _eff_dir(inputs["w_ih_l1r"], inputs["w_hh_l1r"],
                               inputs["b_ih_l1r"], inputs["b_hh_l1r"])

    w = {}
    for g in range(4):
        gs = slice(g * H, (g + 1) * H)
        wx = np.zeros((11, 128), np.float64)
        wx[0:5, 0:64] = Wif.T[:, gs]
        wx[5:10, 64:128] = Wib.T[:, gs]
        wx[10, 0:64] = bf[gs]
        wx[10, 64:128] = bb[gs]
        w[f"wx{g}"] = wx.astype(np.float16)

        rb0 = np.zeros((128, 128), np.float64)
        rb0[0:64, 0:64] = Whf.T[:, gs]
        rb0[64:128, 64:128] = Whb.T[:, gs]
        w[f"rec0_{g}"] = rb0.astype(np.float16)

        rb1 = np.zeros((128, 128), np.float64)
        rb1[0:64, 0:64] = Whf1.T[:, gs]
        rb1[64:128, 64:128] = Whb1.T[:, gs]
        w[f"rec1_{g}"] = rb1.astype(np.float16)

        ia = np.zeros((128, 128), np.float64)
        ia[0:64, 0:64] = Wif1.T[0:64, gs]
        ia[64:128, 64:128] = Wib1.T[64:128, gs]
        w[f"inA{g}"] = ia.astype(np.float16)

        ib = np.zeros((128, 128), np.float64)
        ib[64:128, 0:64] = Wif1.T[64:128, gs]
        ib[0:64, 64:128] = Wib1.T[0:64, gs]
        w[f"inB{g}"] = ib.astype(np.float16)

    bias4 = np.zeros((4, 128), np.float64)
    for g in range(4):
        gs = slice(g * H, (g + 1) * H)
        bias4[g, 0:64] = bf1[gs]
        bias4[g, 64:128] = bb1[gs]
    w["bias4"] = bias4.astype(np.float16)
    mask4 = np.zeros((4, 512), np.float16)
    for g in range(4):
        mask4[g, g * 128:(g + 1) * 128] = 1.0
    w["mask4"] = mask4

    w["fcw"] = np.asarray(inputs["fc_w"], np.float64).T.astype(np.float16)
    w["fcb"] = np.full((B, 1), float(np.asarray(inputs["fc_b"]).reshape(-1)[0]),
                       np.float32)

    x = np.asarray(inputs["x"])

    def core_map(k):
        xc = x[k * B:(k + 1) * B, :T, :]            # [B, T, 5]
        xt = np.ascontiguousarray(xc.transpose(2, 1, 0)).astype(np.float16)  # [5,T,B]
        xos2 = np.empty((11, T * B), np.float16)
        xos2[0:5] = xt.reshape(5, T * B)
        xos2[5:10] = xt[:, ::-1, :].reshape(5, T * B)
        xos2[10] = 1.0
        return {"xos2": xos2, **w}

    return core_map


# ---------------------------------------------------------------- device build

def build_nc(T=FULL_T, num_devices=NCORES):
    nc = bacc.Bacc("TRN2", target_bir_lowering=False, debug=False,
                   num_devices=num_devices)
    xos2_d = nc.dram_tensor("xos2", [11, T * B], F16, kind="ExternalInput")
    wnames = ([f"wx{g}" for g in range(4)]
              + [f"rec0_{g}" for g in range(4)]
              + [f"rec1_{g}" for g in range(4)]
              + [f"inA{g}" for g in range(4)]
              + [f"inB{g}" for g in range(4)])
    wshapes = {n: ([11, 128] if n.startswith("wx") else [128, 128])
               for n in wnames}
    wd = {n: nc.dram_tensor(n, s, F16, kind="ExternalInput")
          for n, s in wshapes.items()}
    bias4_d = nc.dram_tensor("bias4", [4, 128], F16, kind="ExternalInput")
    mask4_d = nc.dram_tensor("mask4", [4, 512], F16, kind="ExternalInput")
    fcw_d = nc.dram_tensor("fcw", [128, 1], F16, kind="ExternalInput")
    fcb_d = nc.dram_tensor("fcb", [B, 1], F32, kind="ExternalInput")
    out_d = nc.dram_tensor("out", [B, 1], F32, kind="ExternalOutput")

    with tile.TileContext(nc) as tc:
        with (
            tc.tile_pool(name="const", bufs=1) as cp,
            tc.tile_pool(name="wk", bufs=4) as wk,
            tc.tile_pool(name="ps", bufs=5, space="PSUM") as pp,
            tc.tile_pool(name="psfc", bufs=1, space="PSUM") as pfc,
        ):
            comb = cp.tile([128, T * B], F16, tag="comb")     # comb_rev
            xos2 = cp.tile([11, T * B], F16, tag="xos2")
            nc.sync.dma_start(xos2[:], xos2_d[:])
            W = {}
            for n in wnames:
                W[n] = cp.tile(wshapes[n], F16, tag=n, name=n)[:]
                nc.sync.dma_start(W[n], wd[n][:])
            bias4 = cp.tile([4, 128], F16, tag="bias4")
            nc.sync.dma_start(bias4[:], bias4_d[:])
            mask4 = cp.tile([4, 512], F16, tag="mask4")
            nc.sync.dma_start(mask4[:], mask4_d[:])
            fcw_s = cp.tile([128, 1], F16, tag="fcw_s")
            nc.sync.dma_start(fcw_s[:], fcw_d[:])
            fcb_s = cp.tile([B, 1], F32, tag="fcb_s")
            nc.sync.dma_start(fcb_s[:], fcb_d[:])

            # per-chain state tiles
            s_st = [cp.tile([128, BC], F32, tag=f"s{c}", name=f"s{c}")
                    for c in range(2)]
            h1 = [cp.tile([128, BC], F16, tag=f"h1_{c}", name=f"h1_{c}")
                  for c in range(2)]
            fcin = cp.tile([128, B], F16, tag="fcin")

            def gcol(g, c):
                return g * 128 + c * BC

            def prefill_l0(t, banks, close=False):
                ps = pp.tile([128, 512], F32, tag="ps", name=f"bank{t}")
                banks[t] = ps
                rhs = xos2[:, t * B:(t + 1) * B]
                for g in range(4):
                    nc.tensor.matmul(ps[:, g * 128:(g + 1) * 128], W[f"wx{g}"],
                                     rhs, start=(g == 0),
                                     stop=(close and g == 3),
                                     skip_group_check=True)

            def prefill_l1(t, banks, close=False):
                ps = pp.tile([128, 512], F32, tag="ps", name=f"l1bank{t}")
                banks[t] = ps
                tf, tb = t, T - 1 - t
                ra = comb[:, tf * B:(tf + 1) * B]
                rb = comb[:, tb * B:(tb + 1) * B]
                for g in range(4):
                    sl = ps[:, g * 128:(g + 1) * 128]
                    nc.tensor.matmul(sl, W[f"inA{g}"], ra, start=(g == 0),
                                     stop=False, skip_group_check=True)
                    nc.tensor.matmul(sl, W[f"inB{g}"], rb, start=False,
                                     stop=False, skip_group_check=True)
                nc.tensor.matmul(ps[:], bias4[:], mask4[:], start=False,
                                 stop=close, skip_group_check=True)

            def rec_mms(layer, t, c, banks):
                # chain 1's recs are the bank's last writes -> stop there
                ps = banks[t]
                if layer == 0:
                    rhs = comb[:, (t - 1) * B + c * BC:(t - 1) * B + (c + 1) * BC]
                    wkey = "rec0_"
                else:
                    rhs = h1[c][:]
                    wkey = "rec1_"
                for g in range(4):
                    nc.tensor.matmul(ps[:, gcol(g, c):gcol(g, c) + BC],
                                     W[f"{wkey}{g}"], rhs, start=False,
                                     stop=(c == 1 and g == 3),
                                     skip_group_check=True)

            def gates_act(t, c, banks, S):
                ps = banks[t]
                gates_ap = ps[:].rearrange("p (g b) -> p g b", g=4)[
                    :, :, c * BC:(c + 1) * BC]
                nc.scalar.activation(S[:], gates_ap, AF.Sigmoid)

            def chain_block(layer, t, c, banks):
                """rec + gates ACT + elementwise for chain c at step t."""
                if t > 0:
                    rec_mms(layer, t, c, banks)
                S = wk.tile([128, 4 * BC], F16, tag=f"S{c}",
                            name=f"S{c}_{layer}_{t}")
                gates_act(t, c, banks, S)
                sg, si = S[:, 2 * BC:3 * BC], S[:, 0:BC]
                pt = wk.tile([128, BC], F16, tag=f"pt{c}",
                             name=f"pt{c}_{layer}_{t}")
                nc.vector.scalar_tensor_tensor(pt[:], sg, 0.5, si,
                                               ALU.subtract, ALU.mult)
                if t == 0:
                    nc.vector.tensor_scalar_mul(s_st[c][:], pt[:], 4.0)
                else:
                    r = wk.tile([128, BC], F32, tag=f"r{c}",
                                name=f"r{c}_{layer}_{t}")
                    nc.vector.tensor_tensor(r[:], S[:, BC:2 * BC],
                                            s_st[c][:], ALU.mult)
                    nc.vector.scalar_tensor_tensor(s_st[c][:], pt[:], 4.0,
                                                   r[:], ALU.mult, ALU.add)
                th = wk.tile([128, BC], F16, tag=f"th{c}",
                             name=f"th{c}_{layer}_{t}")
                nc.scalar.activation(th[:], s_st[c][:], AF.Tanh, scale=0.5)
                if layer == 0:
                    hout = comb[:, t * B + c * BC:t * B + (c + 1) * BC]
                else:
                    hout = h1[c][:]
                nc.gpsimd.tensor_tensor(hout, th[:], S[:, 3 * BC:4 * BC],
                                        ALU.mult)
                if layer == 1 and t == 0:
                    nc.vector.tensor_copy(fcin[64:128, c * BC:(c + 1) * BC],
                                          h1[c][64:128, :])

            def layer_loop(layer, prefill):
                # chain 0 leads; chain 1 trails by one full step, so a bank's
                # writers/readers serialize as: prefill -> rec0 -> gates0 ->
                # rec1 -> gates1 with no cross-chain waiting.
                banks = {}
                prefill(0, banks, close=True)
                prefill(1, banks)
                for step in range(T + 1):
                    if step + 2 <= T - 1:
                        prefill(step + 2, banks)
                    if step < T:
                        chain_block(layer, step, 0, banks)
                    if step >= 1:
                        chain_block(layer, step - 1, 1, banks)
                        banks.pop(step - 1)

            layer_loop(0, prefill_l0)
            layer_loop(1, prefill_l1)
            for c in range(2):
                nc.vector.tensor_copy(fcin[0:64, c * BC:(c + 1) * BC],
                                      h1[c][0:64, :])

            # ================= fc =================
            psf = pfc.tile([128, 1], F32, tag="psf")
            nc.tensor.matmul(psf[:], fcin[:], fcw_s[:], start=True, stop=True)
            outs = wk.tile([B, 1], F32, tag="outs")
            nc.scalar.activation(outs[:], psf[:], AF.Identity, bias=fcb_s[:])
            nc.sync.dma_start(out_d[:], outs[:])

    nc.compile()
    return nc


# ---------------------------------------------------------------- entry points

_NC_CACHE = {}


def _get_nc(T=FULL_T):
    if T not in _NC_CACHE:
        _NC_CACHE[T] = build_nc(T)
    return _NC_CACHE[T]


def kernel(**inputs):
    x = np.asarray(inputs["x"])
    T = x.shape[1]
    nc = _get_nc(T)
    core_map = make_core_inputs(inputs, T)
    in_maps = [core_map(k) for k in range(NCORES)]
    res = run_bass_kernel_spmd(nc, in_maps, list(range(NCORES)),
                               trace=bool(os.environ.get("BASS_TRACE_KERNEL")))
    out = np.concatenate([np.asarray(res.results[k]["out"]) for k in range(NCORES)],
                         axis=0)
    kernel.last_results = res
    return out.astype(np.float32)


# revision 4
# speedup vs baseline: 1.1196x; 1.1196x over previous
"""Trainium2 Bass kernel for nn_BiLSTMModel (2-layer BiLSTM, B=1024 T=256 D=5 H=64).

V2: pure batch-DP over 8 cores (128 samples/core), restructured for engine
overlap:
 - two phase-shifted chains (batch halves of 64) per layer
 - comb_rev storage: col t = [h_f(t) ; h_b(T-1-t)] -> fwd+bwd recurrent matmul
   fuses into ONE block-diag matmul per gate, and the h-write is ONE DVE op
 - bias folded into the x-projection via a ones row (no bias matmul in l0);
   l1 bias via one small K=4 mask matmul (prefilled, off-chain)
 - psum bank per step [128, 512] shared by both chains: col = g*128 + c*64 + b
   prefill (x-proj / l1 in-proj) uses N=128 matmuls covering both chains
 - elementwise split: DVE does pt + s (STT), Pool (gpsimd) does r + h (plain TT;
   Pool has no scalar_tensor_tensor opcode)
 - transformed cell (s = 2c; g-gate rows of W/b scaled 2x so all four gate
   activations are one sigmoid; state nonlinearity via Tanh(0.5*s) so the
   h-write is a plain multiply):
     z  = W@x|h (+b)        -> psum f32
     u  = sigmoid(z)        -> gates ACT, per chain [128, 256] strided read
     pt = (u_g - 0.5)*u_i   -> DVE STT
     r  = u_f*s_old         -> pool TT
     s  = 4*pt + r          -> DVE STT
     th = tanh(0.5*s)       -> state ACT [128, 64] (= tanh(c))
     h  = th*u_o            -> pool TT (full-scale h; no 2x weight folds)
"""
import os
import numpy as np

import concourse.bacc as bacc
import concourse.bass as bass
import concourse.mybir as mybir
import concourse.tile as tile
from concourse.bass_utils import run_bass_kernel_spmd

H = 64
B = 128          # per-core batch
BC = 64          # per-chain batch
NCORES = 8
FULL_T = 256

F16 = mybir.dt.float16
F32 = mybir.dt.float32
AF = mybir.ActivationFunctionType
ALU = mybir.AluOpType


# ---------------------------------------------------------------- host packing

def _eff_dir(w_ih, w_hh, b_ih, b_hh):
    """Transformed-cell effective weights (float64 math). Gate order i,f,g,o.
    Only the g-gate rows are scaled 2x (tanh(g) = 2*sig(2g)-1); h is kept at
    full scale (state nonlinearity = tanh), so no other folds."""
    Wi = np.asarray(w_ih, np.float64).copy()
    Wh = np.asarray(w_hh, np.float64).copy()
    b = (np.asarray(b_ih, np.float64) + np.asarray(b_hh, np.float64)).copy()
    g = slice(2 * H, 3 * H)
    Wi = Wi.copy()
    Wi[g] *= 2.0
    Wh[g] *= 2.0
    b[g] *= 2.0
    return Wi, Wh, b


def make_core_inputs(inputs, T):
    Wif, Whf, bf = _eff_dir(inputs["w_ih_l0"], inputs["w_hh_l0"],
                            inputs["b_ih_l0"], inputs["b_hh_l0"])
    Wib, Whb, bb = _eff_dir(inputs["w_ih_l0r"], inputs["w_hh_l0r"],
                            inputs["b_ih_l0r"], inputs["b_hh_l0r"])
    Wif1, Whf1, bf1 = _eff_dir(inputs["w_ih_l1"], inputs["w_hh_l1"],
                               inputs["b_ih_l1"], inputs["b_hh_l1"])
    Wib1, Whb1, bb1 = _eff_dir(inputs["w_ih_l1r"], inputs["w_hh_l1r"],
                               inputs["b_ih_l1r"], inputs["b_hh_l1r"])

    w = {}
    for g in range(4):
        gs = slice(g * H, (g + 1) * H)
        wx = np.zeros((11, 128), np.float64)
        wx[0:5, 0:64] = Wif.T[:, gs]
        wx[5:10, 64:128] = Wib.T[:, gs]
        wx[10, 0:64] = bf[gs]
        wx[10, 64:128] = bb[gs]
        w[f"wx{g}"] = wx.astype(np.float16)

        rb0 = np.zeros((128, 128), np.float64)
        rb0[0:64, 0:64] = Whf.T[:, gs]
        rb0[64:128, 64:128] = Whb.T[:, gs]
        w[f"rec0_{g}"] = rb0.astype(np.float16)

        rb1 = np.zeros((128, 128), np.float64)
        rb1[0:64, 0:64] = Whf1.T[:, gs]
        rb1[64:128, 64:128] = Whb1.T[:, gs]
        w[f"rec1_{g}"] = rb1.astype(np.float16)

        ia = np.zeros((128, 128), np.float64)
        ia[0:64, 0:64] = Wif1.T[0:64, gs]
        ia[64:128, 64:128] = Wib1.T[64:128, gs]
        w[f"inA{g}"] = ia.astype(np.float16)

        ib = np.zeros((128, 128), np.float64)
        ib[64:128, 0:64] = Wif1.T[64:128, gs]
        ib[0:64, 64:128] = Wib1.T[0:64, gs]
        w[f"inB{g}"] = ib.astype(np.float16)

    bias4 = np.zeros((4, 128), np.float64)
    for g in range(4):
        gs = slice(g * H, (g + 1) * H)
        bias4[g, 0:64] = bf1[gs]
        bias4[g, 64:128] = bb1[gs]
    w["bias4"] = bias4.astype(np.float16)
    mask4 = np.zeros((4, 512), np.float16)
    for g in range(4):
        mask4[g, g * 128:(g + 1) * 128] = 1.0
    w["mask4"] = mask4

    w["fcw"] = np.asarray(inputs["fc_w"], np.float64).T.astype(np.float16)
    w["fcb"] = np.full((B, 1), float(np.asarray(inputs["fc_b"]).reshape(-1)[0]),
                       np.float32)

    x = np.asarray(inputs["x"])

    def core_map(k):
        xc = x[k * B:(k + 1) * B, :T, :]            # [B, T, 5]
        xt = np.ascontiguousarray(xc.transpose(2, 1, 0)).astype(np.float16)  # [5,T,B]
        xos2 = np.empty((11, T * B), np.float16)
        xos2[0:5] = xt.reshape(5, T * B)
        xos2[5:10] = xt[:, ::-1, :].reshape(5, T * B)
        xos2[10] = 1.0
        return {"xos2": xos2, **w}

    return core_map


# ---------------------------------------------------------------- device build

def build_nc(T=FULL_T, num_devices=NCORES):
    nc = bacc.Bacc("TRN2", target_bir_lowering=False, debug=False,
                   num_devices=num_devices)
    xos2_d = nc.dram_tensor("xos2", [11, T * B], F16, kind="ExternalInput")
    wnames = ([f"wx{g}" for g in range(4)]
              + [f"rec0_{g}" for g in range(4)]
              + [f"rec1_{g}" for g in range(4)]
              + [f"inA{g}" for g in range(4)]
              + [f"inB{g}" for g in range(4)])
    wshapes = {n: ([11, 128] if n.startswith("wx") else [128, 128])
               for n in wnames}
    wd = {n: nc.dram_tensor(n, s, F16, kind="ExternalInput")
          for n, s in wshapes.items()}
    bias4_d = nc.dram_tensor("bias4", [4, 128], F16, kind="ExternalInput")
    mask4_d = nc.dram_tensor("mask4", [4, 512], F16, kind="ExternalInput")
    fcw_d = nc.dram_tensor("fcw", [128, 1], F16, kind="ExternalInput")
    fcb_d = nc.dram_tensor("fcb", [B, 1], F32, kind="ExternalInput")
    out_d = nc.dram_tensor("out", [B, 1], F32, kind="ExternalOutput")

    with tile.TileContext(nc) as tc:
        with (
            tc.tile_pool(name="const", bufs=1) as cp,
            tc.tile_pool(name="wk", bufs=4) as wk,
            tc.tile_pool(name="ps", bufs=5, space="PSUM") as pp,
            tc.tile_pool(name="psfc", bufs=1, space="PSUM") as pfc,
        ):
            comb = cp.tile([128, T * B], F16, tag="comb")     # comb_rev
            xos2 = cp.tile([11, T * B], F16, tag="xos2")
            nc.sync.dma_start(xos2[:], xos2_d[:])
            W = {}
            for n in wnames:
                W[n] = cp.tile(wshapes[n], F16, tag=n, name=n)[:]
                nc.sync.dma_start(W[n], wd[n][:])
            bias4 = cp.tile([4, 128], F16, tag="bias4")
            nc.sync.dma_start(bias4[:], bias4_d[:])
            mask4 = cp.tile([4, 512], F16, tag="mask4")
            nc.sync.dma_start(mask4[:], mask4_d[:])
            fcw_s = cp.tile([128, 1], F16, tag="fcw_s")
            nc.sync.dma_start(fcw_s[:], fcw_d[:])
            fcb_s = cp.tile([B, 1], F32, tag="fcb_s")
            nc.sync.dma_start(fcb_s[:], fcb_d[:])

            # per-chain state tiles
            s_st = [cp.tile([128, BC], F32, tag=f"s{c}", name=f"s{c}")
                    for c in range(2)]
            h1 = [cp.tile([128, BC], F16, tag=f"h1_{c}", name=f"h1_{c}")
                  for c in range(2)]
            fcin = cp.tile([128, B], F16, tag="fcin")

            def gcol(g, c):
                return g * 128 + c * BC

            def prefill_l0(t, banks, close=False):
                ps = pp.tile([128, 512], F32, tag="ps", name=f"bank{t}")
                banks[t] = ps
                rhs = xos2[:, t * B:(t + 1) * B]
                for g in range(4):
                    nc.tensor.matmul(ps[:, g * 128:(g + 1) * 128], W[f"wx{g}"],
                                     rhs, start=(g == 0),
                                     stop=(close and g == 3),
                                     skip_group_check=True)

            def prefill_l1(t, banks, close=False):
                ps = pp.tile([128, 512], F32, tag="ps", name=f"l1bank{t}")
                banks[t] = ps
                tf, tb = t, T - 1 - t
                ra = comb[:, tf * B:(tf + 1) * B]
                rb = comb[:, tb * B:(tb + 1) * B]
                for g in range(4):
                    sl = ps[:, g * 128:(g + 1) * 128]
                    nc.tensor.matmul(sl, W[f"inA{g}"], ra, start=(g == 0),
                                     stop=False, skip_group_check=True)
                    nc.tensor.matmul(sl, W[f"inB{g}"], rb, start=False,
                                     stop=False, skip_group_check=True)
                nc.tensor.matmul(ps[:], bias4[:], mask4[:], start=False,
                                 stop=close, skip_group_check=True)

            def rec_mms(layer, t, c, banks):
                # chain 1's recs are the bank's last writes -> stop there
                ps = banks[t]
                if layer == 0:
                    rhs = comb[:, (t - 1) * B + c * BC:(t - 1) * B + (c + 1) * BC]
                    wkey = "rec0_"
                else:
                    rhs = h1[c][:]
                    wkey = "rec1_"
                for g in range(4):
                    nc.tensor.matmul(ps[:, gcol(g, c):gcol(g, c) + BC],
                                     W[f"{wkey}{g}"], rhs, start=False,
                                     stop=(c == 1 and g == 3),
                                     skip_group_check=True)

            def gates_act(t, c, banks, S):
                ps = banks[t]
                gates_ap = ps[:].rearrange("p (g b) -> p g b", g=4)[
                    :, :, c * BC:(c + 1) * BC]
                nc.scalar.activation(S[:], gates_ap, AF.Sigmoid)

            def chain_block(layer, t, c, banks):
                """rec + gates ACT + elementwise for chain c at step t."""
                if t > 0:
                    rec_mms(layer, t, c, banks)
                S = wk.tile([128, 4 * BC], F16, tag=f"S{c}",
                            name=f"S{c}_{layer}_{t}")
                gates_act(t, c, banks, S)
                sg, si = S[:, 2 * BC:3 * BC], S[:, 0:BC]
                pt = wk.tile([128, BC], F16, tag=f"pt{c}",
                             name=f"pt{c}_{layer}_{t}")
                nc.vector.scalar_tensor_tensor(pt[:], sg, 0.5, si,
                                               ALU.subtract, ALU.mult)
                if t == 0:
                    nc.vector.tensor_scalar_mul(s_st[c][:], pt[:], 4.0)
                else:
                    r = wk.tile([128, BC], F32, tag=f"r{c}",
                                name=f"r{c}_{layer}_{t}")
                    nc.vector.tensor_tensor(r[:], S[:, BC:2 * BC],
                                            s_st[c][:], ALU.mult)
                    nc.vector.scalar_tensor_tensor(s_st[c][:], pt[:], 4.0,
                                                   r[:], ALU.mult, ALU.add)
                th = wk.tile([128, BC], F16, tag=f"th{c}",
                             name=f"th{c}_{layer}_{t}")
                nc.scalar.activation(th[:], s_st[c][:], AF.Tanh, scale=0.5)
                if layer == 0:
                    hout = comb[:, t * B + c * BC:t * B + (c + 1) * BC]
                else:
                    hout = h1[c][:]
                nc.vector.tensor_tensor(hout, th[:], S[:, 3 * BC:4 * BC],
                                        ALU.mult)
                if layer == 1 and t == 0:
                    nc.vector.tensor_copy(fcin[64:128, c * BC:(c + 1) * BC],
                                          h1[c][64:128, :])

            def layer_loop(layer, prefill):
                # chain 0 leads; chain 1 trails by one full step, so a bank's
                # writers/readers serialize as: prefill -> rec0 -> gates0 ->
                # rec1 -> gates1 with no cross-chain waiting.
                banks = {}
                prefill(0, banks, close=True)
                prefill(1, banks)
                for step in range(T + 1):
                    if step < T:
                        chain_block(layer, step, 0, banks)
                    # prefill between the chain blocks: the in-order PE queue
                    # then runs it right after rec0, keeping the PE warm so
                    # chain 1's recs avoid the cold-pstate first-matmul penalty
                    if step + 2 <= T - 1:
                        prefill(step + 2, banks)
                    if step >= 1:
                        chain_block(layer, step - 1, 1, banks)
                        banks.pop(step - 1)

            layer_loop(0, prefill_l0)
            layer_loop(1, prefill_l1)
            for c in range(2):
                nc.vector.tensor_copy(fcin[0:64, c * BC:(c + 1) * BC],
                                      h1[c][0:64, :])

            # ================= fc =================
            psf = pfc.tile([128, 1], F32, tag="psf")
            nc.tensor.matmul(psf[:], fcin[:], fcw_s[:], start=True, stop=True)
            outs = wk.tile([B, 1], F32, tag="outs")
            nc.scalar.activation(outs[:], psf[:], AF.Identity, bias=fcb_s[:])
            nc.sync.dma_start(out_d[:], outs[:])

    nc.compile()
    return nc


# ---------------------------------------------------------------- entry points

_NC_CACHE = {}


def _get_nc(T=FULL_T):
    if T not in _NC_CACHE:
        _NC_CACHE[T] = build_nc(T)
    return _NC_CACHE[T]


def kernel(**inputs):
    x = np.asarray(inputs["x"])
    T = x.shape[1]
    nc = _get_nc(T)
    core_map = make_core_inputs(inputs, T)
    in_maps = [core_map(k) for k in range(NCORES)]
    res = run_bass_kernel_spmd(nc, in_maps, list(range(NCORES)),
                               trace=bool(os.environ.get("BASS_TRACE_KERNEL")))
    out = np.concatenate([np.asarray(res.results[k]["out"]) for k in range(NCORES)],
                         axis=0)
    kernel.last_results = res
    return out.astype(np.float32)
